# revision 2
# baseline (speedup 1.0000x reference)
"""Causal multi-head attention (B=4, T=2048, D=1024, H=16) on 8 NeuronCores.

Sharding:
  stage 1 (QKV proj + attention): core c -> batch c//2, head-group c%2
    (8 of 16 heads, 512 of 1024 channels). Data-parallel on B, tensor-
    parallel on heads.
  stage 2 (output projection): one 8-rank AllToAll re-shards attention
    output to (all 4 batches x 256-token t-slice) per core, then each core
    computes out = attn_out @ W_O.T for its 1024 rows. No reduction needed.

Matmul operands are bf16; PSUM accumulation stays fp32. exp runs on the
scalar engine reading PSUM directly with the softmax scale fused; the
softmax denominator comes for free as a 65th output column of the PV
matmul (V augmented with a ones column).

PV orientation: out[q, hd|den] = pt[k, q].T @ (V|1)[k, hd+1] — 65 moving
columns per (head, q-tile, k-tile) instead of the 128/q-tile that a
[hd, q]-oriented PV costs, since PE matmul time scales only with the
moving dim. This halves PV PE time and makes the softmax denominator
per-PARTITION (q), so normalization is one DVE reciprocal + per-partition
tensor_scalar multiplies — no partition broadcast, no PE. PSUM start/stop
is bank-granular (a start marks the whole 2KB zero region pending-zero),
so only each pv bank's first matmul starts and only its last stops; the
four (jt, head) groups inside a bank accumulate independently in between.

The q-major attention output is re-transposed to ch-major for the output
projection by XBAR DMA-transpose instructions (dma_start_transpose,
16x128 tiles, out[p, it, q] = in[q, it*128+p]) reading the
post-collective DRAM buffers — zero PE cost, and the W_O chunk order is
unchanged. The exchange itself stays q-major: [dst j, 128 q, ch] blocks,
written per (chunk, head-pair) as soon as the owning rows normalize.

Scheduling: attention is ACT-bound per k-tile (exp ~1040ns vs QK+PV
~650ns), so projection work for the NEXT chunk is queued and paced into
the attention stream as PE filler. Queue entries carry both a force-drain
label (chunk, kind, index) — consumer sites drain exactly what they
depend on — and a deadline in global attention units; the pacer pops one
due-soon entry per unit (PE executes in emission order, so bursts stall
the exp stream behind them) plus a proportional share of the rest, with a
fixed boost at attend boundaries for the software-pipeline refill bubble.
Chunk-0 is not pre-drained: attend(0,0)'s own force-drains pull
v_block0/q0/k0 just-in-time so exp starts as soon as those land, and the
rest of chunk 0 becomes qc0 filler. V groups queue ahead of hp1-3's Q/K
(their softmax_pv deadline inside attend(0) comes first).

The endgame keeps PE fed through both collectives: the m=0 output
projection and the hp0/1 half of the m=1 projection (split AllToAll) run
as late-qc3 + post-collective filler, a reserve of queued work covers the
final collective+reshard latency, and a chain of scratch matmuls spans
the unavoidable reshard gap so the PE p-state stays at full clock for the
m=1 hp2/3 half (the cost model halves the PE clock after an idle period).
"""
import numpy as np
import ml_dtypes

import concourse.bass as bass
import concourse.mybir as mybir
import concourse.tile as tile
from concourse.bass_utils import run_bass_kernel_spmd

F32 = mybir.dt.float32
BF16 = mybir.dt.bfloat16

P = 128
B, T, D = 4, 2048, 1024
H, HD = 16, 64
NCORES = 8
CH = D // 2          # channels per core (8 heads)
NHP = 4              # head pairs per core
NKT = T // P         # 16 k-tiles
NQC = T // 512       # 4 q-chunks
NIT = D // P         # 8 input-dim tiles


def _split_multiwaits(nc) -> int:
    """walrus here rejects >1 sem wait per instruction; split extras into
    wait-only NoOps on the same engine."""
    nsplit = 0
    for f in nc.m.functions:
        for bb in f.blocks:
            if not any(
                i.sync_info is not None and i.sync_info.on_wait is not None
                and len(i.sync_info.on_wait) > 1 for i in bb.instructions
            ):
                continue
            new_list = []
            for inst in bb.instructions:
                si = inst.sync_info
                if si is not None and si.on_wait is not None and len(si.on_wait) > 1:
                    waits = list(si.on_wait)
                    for k, w in enumerate(waits[:-1]):
                        n = mybir.InstNoOp(
                            name=f"{inst.name}-wsplit{k}", ins=[], outs=[])
                        n.engine = inst.engine
                        n.sync_info = mybir.SyncInfo(on_wait=[w], on_update=[])
                        new_list.append(n)
                        nsplit += 1
                    inst.sync_info = mybir.SyncInfo(
                        on_wait=[waits[-1]], on_update=list(si.on_update or []))
                new_list.append(inst)
            bb.instructions = new_list
    return nsplit


def _build_nc(sim: bool = False):
    nc = bass.Bass("TRN2", target_bir_lowering=False, debug=False,
                   num_devices=NCORES)
    xt_d = nc.dram_tensor("xt", [D, T], BF16, kind="ExternalInput").ap()
    wq_d = nc.dram_tensor("wq", [D, CH], BF16, kind="ExternalInput").ap()
    wk_d = nc.dram_tensor("wk", [D, CH], BF16, kind="ExternalInput").ap()
    wv_d = nc.dram_tensor("wv", [D, CH], BF16, kind="ExternalInput").ap()
    wo_d = nc.dram_tensor("wo", [D, D], BF16, kind="ExternalInput").ap()
    out_d = nc.dram_tensor("out", [B, 2, P, D], BF16,
                       kind="ExternalOutput").ap()
    # q-major exchange buffers: [dst core j, 128 q rows, ch]
    a2a_in0 = nc.dram_tensor("a2a_in0", [NCORES, P, CH], BF16).ap()
    a2a_out0 = nc.dram_tensor("a2a_out0", [NCORES, P, CH], BF16).ap()
    # the m=1 exchange is split by head-pair half: the hp0/1 half fires
    # mid-qc3 (its rows are done once hp1 finishes) so half the m=1 output
    # projection is available as late-qc3 filler; only the hp2/3 half
    # gates the end
    a2a_in1a = nc.dram_tensor("a2a_in1a", [NCORES, P, CH // 2], BF16).ap()
    a2a_out1a = nc.dram_tensor("a2a_out1a", [NCORES, P, CH // 2], BF16).ap()
    a2a_in1b = nc.dram_tensor("a2a_in1b", [NCORES, P, CH // 2], BF16).ap()
    a2a_out1b = nc.dram_tensor("a2a_out1b", [NCORES, P, CH // 2], BF16).ap()

    scale = float(1.0 / np.sqrt(HD))
    # attention units (k-tiles) per q-chunk, for filler pacing
    UNITS = [4 * (qc + 1) * NHP for qc in range(NQC)]

    with tile.TileContext(nc) as tc:
        with (
            tc.tile_pool(name="persist", bufs=1) as persist,
        ):
            # ---- persistent SBUF tensors -------------------------------
            kt_s = persist.tile([P, NHP, T], BF16)    # K^T  (channels, k)
            va = persist.tile([P, NKT, NHP, 2, HD + 1], BF16)  # V | ones

            with (
                tc.tile_pool(name="wpool", bufs=1) as wpool,
                tc.tile_pool(name="xpool", bufs=2) as xpool,
                tc.tile_pool(name="aob_pool", bufs=12) as aob_pool,
                tc.tile_pool(name="osb_pool", bufs=8) as osb_pool,
                tc.tile_pool(name="qpool", bufs=2) as qpool,
                tc.tile_pool(name="ao_pool", bufs=2) as ao_pool,
                tc.tile_pool(name="mpool", bufs=1) as mpool,
                tc.tile_pool(name="pt_pool", bufs=16) as pt_pool,
                tc.tile_pool(name="nrm_pool", bufs=4) as nrm_pool,
                tc.tile_pool(name="ppool", bufs=2, space="PSUM") as ppool,
                tc.tile_pool(name="ps_s", bufs=2, space="PSUM") as ps_s,
                tc.tile_pool(name="ps_pv", bufs=1, space="PSUM") as ps_pv,
            ):
                wq = wpool.tile([P, NIT, CH], BF16)
                wk = wpool.tile([P, NIT, CH], BF16)
                wv = wpool.tile([P, NIT, CH], BF16)
                wo0 = wpool.tile([P, NIT, 512], BF16)
                wo1 = wpool.tile([P, NIT, 512], BF16)
                xt_r = xt_d.rearrange("(i p) t -> p i t", p=P)
                xtc0 = xpool.tile([P, NIT, 512], BF16, tag="xtc")
                # staged arrival: it=0 first (smallest useful unit), then
                # batched remainders — descriptor-heavy DMAs beat
                # instruction-overhead-bound ones (fixed HWDGE cost per
                # DMA), and x/wv interleave because the V matmuls for tile
                # `it` need both tensors' slices and the DMA device is
                # serial
                wv_r = wv_d.rearrange("(i p) o -> p i o", p=P)
                nc.sync.dma_start(xtc0[:, 0], xt_r[:, 0, 0:512])
                nc.sync.dma_start(wv[:, 0], wv_r[:, 0])
                nc.sync.dma_start(xtc0[:, 1:4], xt_r[:, 1:4, 0:512])
                nc.sync.dma_start(wv[:, 1:4], wv_r[:, 1:4])
                nc.sync.dma_start(xtc0[:, 4:], xt_r[:, 4:, 0:512])
                nc.sync.dma_start(wv[:, 4:], wv_r[:, 4:])
                nc.sync.dma_start(wq[:], wq_d.rearrange(
                    "(i p) o -> p i o", p=P))
                nc.sync.dma_start(wk[:], wk_d.rearrange(
                    "(i p) o -> p i o", p=P))

                # ones: broadcast-copy source for the V|ones column
                # (a strided bf16 memset into va fails the ISA memset
                # value-type check, so fill via ACT broadcast copy)
                ones64 = mpool.tile([P, 64], BF16, tag="ones64")
                nc.gpsimd.memset(ones64[:], 1.0)

                nc.scalar.copy(
                    va[:, :, :, :, HD],
                    ones64[:, 0:1].to_broadcast((P, NKT, NHP, 2)))
                # 0/1 causal mask for the 128-col diagonal block: in
                # block-local coords the visible region is q_local >= p for
                # every diagonal tile, so one tile serves all of them
                mask = mpool.tile([P, P], BF16, tag="mask")
                nc.gpsimd.memset(mask[:], 1.0)
                nc.gpsimd.affine_select(
                    out=mask[:], in_=mask[:],
                    compare_op=mybir.AluOpType.is_ge,
                    fill=0.0, base=0, channel_multiplier=-1,
                    pattern=[[1, P]])

                # pending projection psum-groups of upcoming chunks, emitted
                # as PE filler work inside the attention kt loops. Entries
                # are (label, fn) with label=(chunk, kind, idx) so consumer
                # sites can force-drain exactly what they depend on, plus a
                # deadline in global attention units so the pacer pre-pops
                # them just-in-time instead of letting a force-drain dump a
                # multi-group burst that stalls the exp stream.
                INF = 10**9
                UPFX = [0]
                for qc in range(NQC):
                    UPFX.append(UPFX[-1] + UNITS[qc])

                def udl(c, hp, kt):
                    """Global unit index of attend(hp, c)'s k-tile kt."""
                    return UPFX[c] + hp * 4 * (c + 1) + kt

                LOOKAHEAD = 12
                pending = []    # (label, fn, cost_ns, deadline_unit)
                filler_acc = [0.0]
                pcost = [0.0]
                ucur = [0]      # global attention unit counter
                reserve = [0.0]  # ns of work kept for the post-collective gap

                def push(label, fn, cost, deadline=INF):
                    pending.append((label, fn, cost, deadline))
                    pcost[0] += cost

                def pop_front():
                    lb, fn, cost, dl = pending.pop(0)
                    pcost[0] -= cost
                    fn()

                def emit_fillers(remaining_units, boost=0.0):
                    # just-in-time pop: anything due within LOOKAHEAD units
                    # goes now (ahead of its force-drain site), rate-limited
                    # to one per call — PE executes in emission order, so a
                    # burst here delays the attention matmuls behind it by
                    # the whole burst
                    if pending and pending[0][3] <= ucur[0] + LOOKAHEAD:
                        pop_front()
                    # proportional pacing: spread the rest of the queue
                    # across the whole remaining schedule instead of
                    # draining it greedily (late ACT-bound units would idle
                    # PE). `boost` forces extra pops at known PE-stall sites.
                    if not pending or pcost[0] <= reserve[0]:
                        return
                    filler_acc[0] += boost + len(pending) / max(
                        1, remaining_units)
                    while (filler_acc[0] >= 1.0 and pending
                           and pcost[0] > reserve[0]):
                        filler_acc[0] -= 1.0
                        pop_front()

                def force_drain(chunk, kind, idx):
                    """Pop fillers (in order) until no queued entry matches
                    (chunk, kind, <=idx) — consumer is about to read them."""
                    while any(lb[0] == chunk and lb[1] == kind and lb[2] <= idx
                              for lb, _, _, _ in pending):
                        pop_front()

                def project(tc4, xtc=None):
                    """Queue QKV projection psum-groups for t-chunk tc4.
                    Returns the Q^T chunk tile; the groups themselves are
                    emitted later as PE filler inside attention."""
                    if xtc is None:
                        xtc = xpool.tile([P, NIT, 512], BF16, tag="xtc")
                        nc.sync.dma_start(
                            xtc[:], xt_r[:, :, tc4 * 512:(tc4 + 1) * 512])
                    qtc = qpool.tile([P, NHP, 512], BF16, tag="qtc")

                    def qk_group(w, dst, dsl, ot):
                        # two half-contraction pieces sharing one psum tile:
                        # finer filler quanta track the per-k-tile PE deficit
                        # during ACT-bound attention much more closely
                        st = {}

                        def ga():
                            st["ps"] = ppool.tile([P, 512], F32, tag="proj",
                                                  name=f"qk{tc4}_{ot}")
                            for it in range(NIT // 2):
                                nc.tensor.matmul(
                                    st["ps"][:], w[:, it, ot * P:(ot + 1) * P],
                                    xtc[:, it], start=(it == 0), stop=False)

                        def gb():
                            ps = st["ps"]
                            for it in range(NIT // 2, NIT):
                                nc.tensor.matmul(
                                    ps[:], w[:, it, ot * P:(ot + 1) * P],
                                    xtc[:, it], start=False,
                                    stop=(it == NIT - 1))
                            nc.vector.tensor_copy(dst[:, ot, dsl], ps[:])
                        return ga, gb

                    def v_group(tt4):
                        st = {}

                        def ga():
                            st["ps"] = ppool.tile([P, 512], F32, tag="proj",
                                                  name=f"v{tc4}_{tt4}")
                            for it in range(NIT // 2):
                                nc.tensor.matmul(
                                    st["ps"][:],
                                    xtc[:, it, tt4 * P:(tt4 + 1) * P],
                                    wv[:, it], start=(it == 0), stop=False)

                        def gb():
                            ps = st["ps"]
                            for it in range(NIT // 2, NIT):
                                nc.tensor.matmul(
                                    ps[:], xtc[:, it, tt4 * P:(tt4 + 1) * P],
                                    wv[:, it], start=False,
                                    stop=(it == NIT - 1))
                            nc.vector.tensor_copy(
                                va[:, tc4 * 4 + tt4, :, :, 0:HD],
                                ps[:].rearrange("p (hp h d) -> p hp h d",
                                                hp=NHP, h=2))
                        return ga, gb

                    if tc4 == 0:
                        # V first: wv+x arrive first and the four V groups
                        # run it-major across four concurrent psums, so each
                        # arriving (x, wv) DMA chunk feeds 4 matmuls instead
                        # of 1 during the DMA-bound startup ramp
                        def v_block0():
                            pss = [
                                ppool.tile([P, 512], F32, tag="proj",
                                           name="v0ps0"),
                                ppool.tile([P, 512], F32, tag="proj",
                                           name="v0ps1"),
                                ps_s.tile([P, 512], F32, tag="s2",
                                          name="v0ps2"),
                                ps_s.tile([P, 512], F32, tag="s2",
                                          name="v0ps3"),
                            ]
                            for it in range(NIT):
                                for tt4 in range(4):
                                    nc.tensor.matmul(
                                        pss[tt4][:],
                                        xtc[:, it, tt4 * P:(tt4 + 1) * P],
                                        wv[:, it], start=(it == 0),
                                        stop=(it == NIT - 1))
                            for tt4 in range(4):
                                nc.vector.tensor_copy(
                                    va[:, tt4, :, :, 0:HD],
                                    pss[tt4][:].rearrange(
                                        "p (hp h d) -> p hp h d",
                                        hp=NHP, h=2))
                        push((0, "v", 3), v_block0, 6816, udl(0, 0, 0))
                        for ot in range(NHP):
                            for g in qk_group(wq, qtc, slice(0, 512), ot):
                                push((0, "q", ot), g, 853, udl(0, ot, 0))
                            for g in qk_group(
                                    wk, kt_s,
                                    slice(tc4 * 512, (tc4 + 1) * 512), ot):
                                push((0, "k", ot), g, 853, udl(0, ot, 0))
                    else:
                        # queue order mirrors consumption deadlines so
                        # force-drains pop the minimum prefix: q/k of hp0
                        # (attend(0) start + its chunk-tc4 k-tiles), then V
                        # (softmax_pv of those same k-tiles, still hp0),
                        # then q/k of hp1..3
                        for ot in range(NHP):
                            for g in qk_group(wq, qtc, slice(0, 512), ot):
                                push((tc4, "q", ot), g, 853, udl(tc4, ot, 0))
                            for g in qk_group(
                                    wk, kt_s,
                                    slice(tc4 * 512, (tc4 + 1) * 512), ot):
                                push((tc4, "k", ot), g, 853,
                                     udl(tc4, ot, 4 * tc4))
                            if ot == 0:
                                for tt4 in range(4):
                                    for g in v_group(tt4):
                                        push((tc4, "v", tt4), g, 853,
                                             udl(tc4, 0, 4 * tc4 + tt4))
                    return qtc

                def attend(hp, qc, qtc, aoq, rem_after, rem_chunk):
                    """Attention for head-pair hp, q-chunk qc. kt loop is
                    software-pipelined: QK(kt+1) issues before PV(kt) so PE
                    isn't stalled behind the exp of the current tile.

                    PV is q-partitioned: psum group (h, jt) accumulates
                    out[q, hd|den] over k-tiles; group jt stops at its
                    diagonal k-tile, after which it is normalized (DVE
                    recip + per-partition scalar muls) into aoq."""
                    nkt = 4 * (qc + 1)
                    final = hp == NHP - 1 and qc in (1, 3)
                    force_drain(qc, "q", hp)
                    # [q, jt-in-tile, h, hd|den]; padded to 128 f32 = one
                    # whole 2KB PSUM bank so the second tile stays
                    # bank-aligned and no accumulation slice crosses a bank
                    pv0 = ps_pv.tile([P, 2, 2, P], F32, tag="pv0")
                    pv1 = ps_pv.tile([P, 2, 2, P], F32, tag="pv1")
                    pvt = [pv0, pv1]
                    s2s = {}

                    def qk(kt):
                        force_drain(kt // 4, "k", hp)
                        ksl = slice(kt * P, (kt + 1) * P)
                        f0 = max(0, kt - 4 * qc) * P  # first visible q col
                        s2 = ps_s.tile([P, 1024], F32, tag="s2")
                        nc.tensor.matmul(s2[:, f0:512], kt_s[0:64, hp, ksl],
                                         qtc[0:64, hp, f0:],
                                         start=True, stop=True)
                        nc.tensor.matmul(s2[:, 512 + f0:1024],
                                         kt_s[64:128, hp, ksl],
                                         qtc[64:128, hp, f0:],
                                         start=True, stop=True)
                        s2s[kt] = s2

                    def normalize(t):
                        """Both jt groups of psum tile t are stopped: divide
                        by the denominator column into aoq (bf16, q-major)."""
                        rden = nrm_pool.tile([P, 2, 2, 1], F32, tag="rden")
                        nc.vector.reciprocal(
                            rden[:], pvt[t][:, :, :, HD:HD + 1])
                        for jtl in range(2):
                            jt = 2 * t + jtl
                            for h in range(2):
                                nc.vector.tensor_scalar_mul(
                                    aoq[:, jt, hp, h],
                                    pvt[t][:, jtl, h, 0:HD],
                                    rden[:, jtl, h])
                        # ship finished q-row blocks to the exchange buffers
                        # as soon as the last head-pair has normalized them,
                        # so the collective launch isn't gated on one burst
                        for jtl in range(2):
                            jt = 2 * t + jtl
                            j = (qc % 2) * 4 + jt
                            if qc < 2:
                                if hp == NHP - 1:
                                    nc.sync.dma_start(
                                        a2a_in0[j], aoq[:, jt].rearrange(
                                            "p hp h d -> p (hp h d)"))
                            else:
                                if hp == 1:
                                    nc.sync.dma_start(
                                        a2a_in1a[j],
                                        aoq[:, jt, 0:2].rearrange(
                                            "p hp h d -> p (hp h d)"))
                                elif hp == NHP - 1:
                                    nc.sync.dma_start(
                                        a2a_in1b[j],
                                        aoq[:, jt, 2:4].rearrange(
                                            "p hp h d -> p (hp h d)"))

                    def softmax_pv(kt, remaining):
                        ucur[0] += 1
                        force_drain(kt // 4, "v", kt % 4)
                        s2 = s2s.pop(kt)
                        pt = pt_pool.tile([P, 2, 512], BF16, tag="pt")
                        di = kt - 4 * qc
                        # diagonal blocks: only columns >= f0 are causally
                        # visible; exp and PV restrict to them (kt==0 is
                        # always full-width, initializing every PSUM column
                        # of the PV accumulators).
                        f0 = max(0, di) * P
                        s2v = s2[:].rearrange("p (a b) -> p a b", a=2)
                        if f0 > 0:
                            nc.scalar.activation(
                                pt[:, :, f0:], s2v[:, :, f0:],
                                mybir.ActivationFunctionType.Exp,
                                scale=scale)
                        else:
                            nc.scalar.activation(
                                pt[:].rearrange("p a b -> p (a b)"), s2[:],
                                mybir.ActivationFunctionType.Exp,
                                scale=scale)
                        if di >= 0:
                            nc.vector.tensor_mul(
                                pt[:, :, f0:f0 + P], pt[:, :, f0:f0 + P],
                                mask[:, None, :].to_broadcast((P, 2, P)))
                        # filler ahead of the PV group in program order: if
                        # PV head-of-line-blocks on this tile's exp, the
                        # filler already in the queue runs during the wait
                        emit_fillers(remaining)
                        # PSUM start/stop semantics are bank-granular: a
                        # start marks the whole 2KB zero region pending-zero
                        # (each group's bytes then zero on first touch), so
                        # only the bank's FIRST matmul starts and only its
                        # LAST stops; the four (jt%2, h) groups inside a
                        # bank accumulate independently in between.
                        for jt in range(4):
                            if di > jt:
                                continue  # group already stopped
                            for h in range(2):
                                nc.tensor.matmul(
                                    pvt[jt // 2][:, jt % 2, h, 0:HD + 1],
                                    pt[:, h, jt * P:(jt + 1) * P],
                                    va[:, kt, hp, h],
                                    start=(kt == 0 and h == 0
                                           and jt % 2 == 0),
                                    stop=(di == jt and jt % 2 == 1
                                          and h == 1))
                        if di == 1:
                            normalize(0)
                        if di == 3:
                            normalize(1)
                        if final and kt >= nkt - 2 and pending:
                            # the reserve normally blocks pops here, but on
                            # a collective-gating attend the diagonal
                            # cascade idles PE and the held work runs
                            # before the collective either way — spend it
                            # at the stall sites
                            pop_front()

                    def rem(left):
                        return left + rem_after

                    qk(0)
                    for kt in range(1, nkt):
                        qk(kt)
                        if kt == 1:
                            # cross-attend boundary: PV(0) waits on exp(0)
                            # and qk(2) on the s2 slot it frees — nothing
                            # attention-side can run, so force one filler in
                            emit_fillers(rem(nkt - 1), boost=1.4)
                        softmax_pv(kt - 1, rem(nkt - kt))
                    softmax_pv(nkt - 1, rem(1))

                def emit_collective(cin, cout):
                    if sim:
                        nc.sync.dma_start(cout, cin)
                    else:
                        nc.gpsimd.collective_compute(
                            "AllToAll", mybir.AluOpType.bypass,
                            replica_groups=[list(range(NCORES))],
                            ins=[cin], outs=[cout])

                # XBAR-transpose a received q-major block back to ch-major:
                # in [128 q, n*128 ch] DRAM -> out[p, it, q] = in[q, it*128+p]
                # so aob chunk it holds channels [it*128, (it+1)*128) — the
                # same chunk order the W_O tiles already use.
                def transpose_aob(aob, cslice, src):
                    nc.sync.dma_start(aob[:, cslice], src, transpose=True)

                def o_stage(b, m, aob_get):
                    """Four labeled half-group filler entries (two per
                    512-col half of W_O) sharing one staging tile."""
                    slot = {}

                    def g(oc, half):
                        def f():
                            if "osb" not in slot:
                                slot["osb"] = osb_pool.tile(
                                    [P, D], BF16, tag="osb",
                                    name=f"osb{m}_{b}")
                            w = (wo0, wo1)[oc]
                            aob = aob_get(b)
                            if half == 0:
                                slot[oc] = ppool.tile(
                                    [P, 512], F32, tag="proj",
                                    name=f"og{m}_{b}_{oc}")
                                for ct in range(NIT // 2):
                                    nc.tensor.matmul(
                                        slot[oc][:], aob[:, ct], w[:, ct],
                                        start=(ct == 0), stop=False)
                                return
                            ps = slot.pop(oc)
                            for ct in range(NIT // 2, NIT):
                                nc.tensor.matmul(
                                    ps[:], aob[:, ct], w[:, ct],
                                    start=False, stop=(ct == NIT - 1))
                            osl = slice(oc * 512, (oc + 1) * 512)
                            nc.vector.tensor_copy(slot["osb"][:, osl], ps[:])
                            if oc == 1:
                                nc.sync.dma_start(out_d[b, m],
                                                  slot["osb"][:])
                        return f
                    return [((9, "o", 2 * b + oc), g(oc, half), 853)
                            for oc in range(2) for half in range(2)]

                # m=1 output projection in two channel-halves: the hp0/1
                # contraction (W_O chunks {0,1,4,5}) runs off collective 1a
                # as late-qc3 filler; the hp2/3 half accumulates on top
                # after collective 1b
                osb1 = {}
                HALF_A = (0, 1, 4, 5)
                HALF_B = (2, 3, 6, 7)

                def o_half(b, aob, oc, cts, first):
                    # aob holds the half's 4 chunks at local indices i in
                    # the same order as the global chunk list cts
                    ps = ppool.tile([P, 512], F32, tag="proj")
                    for i, ct in enumerate(cts):
                        nc.tensor.matmul(
                            ps[:], aob[:, i], (wo0, wo1)[oc][:, ct],
                            start=(i == 0), stop=(i == len(cts) - 1))
                    osl = slice(oc * 512, (oc + 1) * 512)
                    if first:
                        nc.vector.tensor_copy(osb1[b][:, osl], ps[:])
                    else:
                        nc.vector.tensor_add(osb1[b][:, osl],
                                             osb1[b][:, osl], ps[:])
                        nc.sync.dma_start(out_d[b, 1, :, osl],
                                          osb1[b][:, osl])

                def o1a_stage(b, aob_get):
                    def g(oc):
                        def f():
                            if b not in osb1:
                                osb1[b] = osb_pool.tile(
                                    [P, D], BF16, tag="osb",
                                    name=f"osb1_{b}")
                            o_half(b, aob_get(b), oc, HALF_A, True)
                        return f
                    return [((9, "oa", 2 * b + oc), g(oc), 852)
                            for oc in range(2)]

                aob0 = {}
                aob1a = {}
                aob1b = {}
                qtc = project(0, xtc=xtc0)
                # no pre-drain: attend(0,0)'s force-drains pull v_block0 +
                # q0/k0 just-in-time, so exp starts as soon as those land;
                # chunk 0's hp1-3 projections become qc0 fillers instead of
                # serializing ahead of the first attend
                for tc4 in range(NQC):
                    if tc4 + 1 < NQC:
                        next_qtc = project(tc4 + 1)  # queued as fillers
                    aoq = ao_pool.tile([P, 4, NHP, 2, HD], BF16, tag="aoq")
                    for hp in range(NHP):
                        if tc4 == 3 and hp == 2:
                            # hp0/1 rows of both m=1 chunks are shipped:
                            # fire the first m=1 half-collective and queue
                            # its output projection as late-qc3 filler
                            emit_collective(a2a_in1a, a2a_out1a)
                            for b in range(B):
                                aob1a[b] = aob_pool.tile(
                                    [P, NIT // 2, P], BF16, tag="aob",
                                    name=f"aob1a_{b}")
                                for g in range(2):
                                    transpose_aob(
                                        aob1a[b],
                                        slice(g * 2, (g + 1) * 2),
                                        a2a_out1a[2 * b + g])
                            for b in range(B):
                                for lb, fn, cost in o1a_stage(b, aob1a.get):
                                    push(lb, fn, cost)
                            reserve[0] = 15500
                        rem_chunk = (NHP - 1 - hp) * 4 * (tc4 + 1)
                        rem_after = rem_chunk + sum(UNITS[tc4 + 1:])
                        attend(hp, tc4, qtc, aoq, rem_after, rem_chunk)
                    if tc4 == 1:
                        emit_collective(a2a_in0, a2a_out0)
                    if tc4 == 2:
                        # qc0/qc1 results finished resharding during qc2:
                        # W_O streams in now and the received q-major blocks
                        # are XBAR-transposed back to ch-major. The m=0
                        # output projection joins the filler queue behind
                        # chunk-3's projections, but a reserve of groups is
                        # held back so PE has work after collective 1a is
                        # emitted, hiding the collective+reshard latency
                        # that gates m=1.
                        wo_r = wo_d.rearrange("(i p) o -> p i o", p=P)
                        nc.sync.dma_start(wo0[:], wo_r[:, :, 0:512])
                        nc.sync.dma_start(wo1[:], wo_r[:, :, 512:1024])
                        for b in range(B):
                            aob0[b] = aob_pool.tile(
                                [P, NIT, P], BF16, tag="aob",
                                name=f"aob0_{b}")
                            for g in range(2):
                                transpose_aob(
                                    aob0[b], slice(g * 4, (g + 1) * 4),
                                    a2a_out0[2 * b + g])
                        for b in range(B):
                            for lb, fn, cost in o_stage(b, 0, aob0.get):
                                push(lb, fn, cost)
                        reserve[0] = 15500
                    if tc4 + 1 < NQC:
                        qtc = next_qtc
                # release most of the reserve: from here the remaining
                # queue runs after the collective emission in program
                # order, so holding more than the collective+reshard
                # latency only delays the m=1 hp2/3 half
                reserve[0] = 0
                while pending:
                    pop_front()
                emit_collective(a2a_in1b, a2a_out1b)

                # m=1b reshard transposes issue first: they only wait on
                # the collective, and queueing them behind other work would
                # delay them on the in-order HWDGE queue
                for b in range(B):
                    aob1b[b] = aob_pool.tile(
                        [P, NIT // 2, P], BF16, tag="aob",
                        name=f"aob1b_{b}")
                    for g in range(2):
                        transpose_aob(aob1b[b], slice(g * 2, (g + 1) * 2),
                                      a2a_out1b[2 * b + g])

                # reserved groups hide the collective+reshard latency
                reserve[0] = 0.0
                while pending:
                    pop_front()

                # keep the PE p-state ramped across the collective+reshard
                # gap: the cost model (like the hardware) drops the PE
                # clock after an idle period, which would run the whole
                # m=1 hp2/3 projection at roughly half speed. A chain of
                # scratch matmuls (overwriting a dead s2 bank) spans the
                # gap; they are pure filler with no readers.
                for _ in range(30):
                    dps = ps_s.tile([P, 512], F32, tag="s2", name="dps")
                    nc.tensor.matmul(dps[:], wq[0:64, 0, 0:P], wq[0:64, 1],
                                     start=True, stop=True)

                # ---- m=1 hp2/3 half: accumulate + store ----------------
                for b in range(B):
                    for oc in range(2):
                        o_half(b, aob1b[b], oc, HALF_B, False)

    _split_multiwaits(nc)
    return nc


_NC_CACHE = None


def _get_nc():
    global _NC_CACHE
    if _NC_CACHE is None:
        _NC_CACHE = _build_nc()
    return _NC_CACHE


def make_in_maps(x, W_Q, W_K, W_V, W_O):
    bf = ml_dtypes.bfloat16
    wqt = np.ascontiguousarray(W_Q.T).astype(bf)
    wkt = np.ascontiguousarray(W_K.T).astype(bf)
    wvt = np.ascontiguousarray(W_V.T).astype(bf)
    wot = np.ascontiguousarray(W_O.T).astype(bf)
    in_maps = []
    for c in range(NCORES):
        b, g = c // 2, c % 2
        in_maps.append({
            "xt": np.ascontiguousarray(x[b].T).astype(bf),
            "wq": np.ascontiguousarray(wqt[:, g * CH:(g + 1) * CH]),
            "wk": np.ascontiguousarray(wkt[:, g * CH:(g + 1) * CH]),
            "wv": np.ascontiguousarray(wvt[:, g * CH:(g + 1) * CH]),
            "wo": wot,
        })
    return in_maps


def assemble(results):
    out = np.empty((B, T, D), np.float32)
    for j in range(NCORES):
        o = np.asarray(results[j]["out"], np.float32)  # [B, 2, 128, D]
        for b in range(B):
            for m in range(2):
                r0 = m * 1024 + j * P
                out[b, r0:r0 + P, :] = o[b, m]
    return out


def kernel(x, W_Q, W_K, W_V, W_O):
    x = np.asarray(x, np.float32)
    in_maps = make_in_maps(x, np.asarray(W_Q, np.float32),
                           np.asarray(W_K, np.float32),
                           np.asarray(W_V, np.float32),
                           np.asarray(W_O, np.float32))
    nc = _get_nc()
    res = run_bass_kernel_spmd(nc, in_maps, core_ids=list(range(NCORES)))
    return assemble(res.results)
